# revision 3
# baseline (speedup 1.0000x reference)
"""AttentionBlock Trainium2 kernel (B=4, N=2048, C=1024, H=16, D=64, EMB=1024).

    se = emb @ W_emb.T + b_emb;  scale, shift = split(se, 2, -1)
    h  = LN(x) * (1+scale) + shift
    q,k,v = split(h @ W_proj.T) -> (B,H,N,D);  q = LN(q); k = LN(k)  (over D)
    o  = softmax(q k^T / sqrt(D)) v  -> (B,N,C)
    out = o + o @ W_out.T

Sharding: 8 cores; core c -> batch b=c//2, query-half j=c%2. The host rolls
the token axis per core so its query tokens are always tokens 0:1024
(attention is permutation-equivariant over key/value tokens), giving one
symmetric SPMD NEFF with no collectives. Each core computes the full-batch
preamble (se/h/k/v over all 2048 tokens), and q/attention/out-proj for its
1024 rows.

Dataflow is feature-major (channels on partitions) end to end:
  - LayerNorm over channels == partition reduction -> ones-column matmuls.
  - Per-token (free-dim) scalars broadcast across partitions by bouncing a
    row through DRAM (DRAM APs allow step-0 partition dims; SBUF APs don't).
  - q/k LN centering rides the score matmul as an augmented 65th row
    (k_aug row64 = 8*mu_k[m], q_aug row64 = -8*mu_q[n]*rq[n]); the rk[m]/8
    factor is applied by the ACT exp per-partition scale operand.
  - Softmax denominators come free as a ones column appended to v; the
    division is deferred until after the attn@v matmul.
  - The residual is folded into the output projection: W_res = (I+W_out).T.

Matmuls run in float32r (1 cycle/row at free-dim>=256, ~1.5e-4 rel err);
attention probabilities/values use bf16 (errors cancel in the softmax ratio).
"""

import sys

sys.path.insert(0, "/opt/trn_rl_repo")

import numpy as np

import concourse.bass as bass
import concourse.mybir as mybir
import concourse.tile as tile
from concourse import bacc
from concourse.bass_utils import run_bass_kernel_spmd

P = 128
B, N, C = 4, 2048, 1024
H, D = 16, 64
EMB = 1024
EPS = 1e-5
T = N          # tokens per batch on each core (k/v coverage)
TQ = N // 2    # query tokens per core
CH = C // P    # 8 channel chunks
O2 = 2 * C
NCORES = 8
TT = 256       # token tile in phase A1
NTT = T // TT
NMT = T // 512   # 4  key-token tiles (512)
NMC = T // P     # 16 key-token chunks (128)
NNT = TQ // 512  # 2  query-token tiles (512)

F32 = mybir.dt.float32
F32R = mybir.dt.float32r
BF16 = mybir.dt.bfloat16
MUL = mybir.AluOpType.mult
ADD = mybir.AluOpType.add
SUB = mybir.AluOpType.subtract
EXP = mybir.ActivationFunctionType.Exp
SQRT = mybir.ActivationFunctionType.Sqrt

_cached = {}


def build_kernel(debug=None):
    nc = bacc.Bacc()

    xT = nc.dram_tensor("xT", [C, T], F32R, kind="ExternalInput")
    embT = nc.dram_tensor("embT", [EMB, T], F32R, kind="ExternalInput")
    WembT = nc.dram_tensor("WembT", [EMB, O2], F32R, kind="ExternalInput")
    bemb = nc.dram_tensor("bemb", [P, O2 // P], F32, kind="ExternalInput")
    WprojT = nc.dram_tensor("WprojT", [C, 3 * C], F32R, kind="ExternalInput")
    WresT = nc.dram_tensor("WresT", [C, C], F32R, kind="ExternalInput")
    out = nc.dram_tensor("out", [TQ, C], F32, kind="ExternalOutput")

    xT_r = xT.rearrange("(ch p) t -> p ch t", p=P)
    embT_r = embT.rearrange("(ch p) t -> p ch t", p=P)
    WembT_r = WembT.rearrange("(ch p) o -> p ch o", p=P)
    WprojT_r = WprojT.rearrange("(ch p) o -> p ch o", p=P)
    WresT_r = WresT.rearrange("(ch p) o -> p ch o", p=P)

    with tile.TileContext(nc) as tc:
        with (
            tc.tile_pool(name="const", bufs=1) as const,
            tc.tile_pool(name="main", bufs=1) as main,
            tc.tile_pool(name="dram", bufs=2, space="DRAM") as dram,
            tc.tile_pool(name="ps_mm", bufs=3, space="PSUM") as ps_mm,
            tc.tile_pool(name="ps_ot", bufs=2, space="PSUM") as ps_ot,
            tc.tile_pool(name="ps_row", bufs=3, space="PSUM") as ps_row,
        ):
            # ---------------- constants ----------------
            eps_t = const.tile([P, 1], F32, name="eps_t")
            nc.vector.memset(eps_t[:], EPS)
            # memset can't emit float32r: stage constants in F32, copy-round.
            cscr = const.tile([P, 65], F32, name="cscr")
            ones_col = const.tile([P, 1], F32R, name="ones_col")
            nc.vector.memset(cscr[:, 0:1], 1.0)
            nc.vector.tensor_copy(ones_col[:], cscr[:, 0:1])
            # per-head partition-block sums: lhsT cols 0 and 64 select heads,
            # so the row-matmul output lands at partitions 0 and 64 (aligned).
            bo8 = const.tile([P, 65], F32R, name="bo8")      # +1/8
            bon8 = const.tile([P, 65], F32R, name="bon8")    # -1/8
            bo64 = const.tile([P, 65], F32R, name="bo64")    # +1/64
            for t_, v_ in ((bo8, 0.125), (bon8, -0.125), (bo64, 1.0 / 64)):
                nc.vector.memset(cscr[:], 0.0)
                nc.vector.memset(cscr[0:64, 0:1], v_)
                nc.vector.memset(cscr[64:128, 64:65], v_)
                nc.vector.tensor_copy(t_[:], cscr[:])
            bemb_sb = const.tile([P, O2 // P], F32, name="bemb_sb")
            nc.sync.dma_start(bemb_sb[:], bemb[:])

            h_sb = main.tile([P, CH, T], F32R, name="h_sb")  # 64KB/part
            o_fm = main.tile([P, CH, TQ], F32R, name="o_fm")  # 32KB/part

            # ============ Phase A1: se + LN(x) + FiLM -> h ============
            with (
                tc.tile_pool(name="wembp", bufs=1) as wembp,
                tc.tile_pool(name="a1s", bufs=2) as a1s,
                tc.tile_pool(name="a1r", bufs=2) as a1r,
            ):
                wemb_sb = wembp.tile([P, CH, O2], F32R, name="wemb_sb")
                nc.sync.dma_start(wemb_sb[:], WembT_r)

                for tt in range(NTT):
                    tsl = slice(tt * TT, (tt + 1) * TT)
                    x_t = a1s.tile([P, CH, TT], F32R, name="x_t")
                    nc.sync.dma_start(x_t[:], xT_r[:, :, tsl])
                    e_t = a1s.tile([P, CH, TT], F32R, name="e_t")
                    nc.sync.dma_start(e_t[:], embT_r[:, :, tsl])

                    # LN stats over channels (partition reduction via matmul)
                    ps_s = ps_row.tile([2, 512], F32, name="ps_s", tag="ps_row")
                    ps_s2 = ps_row.tile([2, 512], F32, name="ps_s2", tag="ps_row")
                    for ch in range(CH):
                        x2c = a1r.tile([P, TT], F32R, name="x2c", tag="scr")
                        nc.scalar.square(x2c[:], x_t[:, ch, :])
                        nc.tensor.matmul(ps_s[0:1, 0:TT], ones_col[:], x_t[:, ch, :],
                                         start=(ch == 0), stop=(ch == CH - 1))
                        nc.tensor.matmul(ps_s2[0:1, 0:TT], ones_col[:], x2c[:],
                                         start=(ch == 0), stop=(ch == CH - 1))
                    mu = a1r.tile([1, TT], F32, name="mu")
                    m2 = a1r.tile([1, TT], F32, name="m2")
                    vr = a1r.tile([1, TT], F32, name="vr")
                    nmr = a1r.tile([1, TT], F32, name="nmr")
                    nc.vector.tensor_scalar_mul(mu[:], ps_s[0:1, 0:TT], 1.0 / C)
                    nc.vector.tensor_tensor(m2[:], mu[:], mu[:], MUL)  # mu^2
                    # var = ps_s2/C - mu^2  (one input in PSUM, one SBUF)
                    nc.vector.scalar_tensor_tensor(vr[:], ps_s2[0:1, 0:TT], 1.0 / C, m2[:],
                                                   mybir.AluOpType.mult, SUB)
                    nc.scalar.activation(vr[:], vr[:], SQRT, bias=eps_t[0:1], scale=1.0)
                    nc.vector.reciprocal(vr[:], vr[:])          # rstd
                    # nmr = -mu * rstd
                    nc.vector.scalar_tensor_tensor(nmr[:], mu[:], -1.0, vr[:], MUL, MUL)
                    rstd = vr[:]

                    rows_d = dram.tile([2, TT], F32, name="rows_d")
                    nc.sync.dma_start(rows_d[0:1, :], rstd)
                    nc.sync.dma_start(rows_d[1:2, :], nmr[:])
                    rstd_bc = a1r.tile([P, TT], F32, name="rstd_bc")
                    nc.sync.dma_start(rstd_bc[:], rows_d[0:1, :].to_broadcast((P, TT)))
                    nmr_bc = a1r.tile([P, TT], F32, name="nmr_bc")
                    nc.sync.dma_start(nmr_bc[:], rows_d[1:2, :].to_broadcast((P, TT)))

                    for ch in range(CH):
                        ps_sc = ps_mm.tile([P, 512], F32, name="ps_sc", tag="ps_mm")
                        for ech in range(CH):
                            nc.tensor.matmul(ps_sc[:, 0:TT],
                                             wemb_sb[:, ech, ch * P:(ch + 1) * P],
                                             e_t[:, ech, :],
                                             start=(ech == 0), stop=(ech == CH - 1))
                        ps_sh = ps_mm.tile([P, 512], F32, name="ps_sh", tag="ps_mm")
                        for ech in range(CH):
                            nc.tensor.matmul(ps_sh[:, 0:TT],
                                             wemb_sb[:, ech, C + ch * P:C + (ch + 1) * P],
                                             e_t[:, ech, :],
                                             start=(ech == 0), stop=(ech == CH - 1))
                        nc.vector.tensor_scalar_add(ps_sc[:, 0:TT], ps_sc[:, 0:TT], bemb_sb[:, ch:ch + 1])
                        nc.vector.tensor_scalar_add(ps_sh[:, 0:TT], ps_sh[:, 0:TT], bemb_sb[:, CH + ch:CH + ch + 1])
                        xn = a1r.tile([P, TT], F32, name="xn", tag="scr")
                        nc.vector.tensor_tensor(xn[:], x_t[:, ch, :], rstd_bc[:], MUL)
                        nc.vector.tensor_tensor(xn[:], xn[:], nmr_bc[:], ADD)
                        nc.vector.tensor_tensor(xn[:], xn[:], ps_sc[:, 0:TT], MUL)
                        nc.vector.tensor_tensor(h_sb[:, ch, tsl], xn[:], ps_sh[:, 0:TT], ADD)

            if debug == "h":
                # dump h (first TQ tokens) to out: out[t, c] = h[c, t]
                # dump h feature-major: out viewed as [C, TQ]
                nc.gpsimd.dma_start(out.rearrange("(ch p) t -> p ch t", p=P),
                                    h_sb[:, :, 0:TQ])
            if debug != "h":
                # ============ Phase B: per-head-pair qkv + attention ============
                with (
                    tc.tile_pool(name="bw", bufs=1) as bw,
                    tc.tile_pool(name="batt", bufs=1) as batt,
                tc.tile_pool(name="bp", bufs=2) as bp,
                    tc.tile_pool(name="bsm", bufs=1) as bsm,
                    tc.tile_pool(name="bq", bufs=2) as bq,
                ):
                    _dbg_b = debug in ("ka", "qa", "p", "b1")
                    do_q = debug not in ("ka",)
                    do_sc = debug not in ("ka", "qa")
                    do_o = debug not in ("ka", "qa", "p")
                    for hq in range(1 if _dbg_b else 4):  # head quads
                        wv_sb = bw.tile([P, CH, 256], F32R, name="wv_sb")
                        nc.sync.dma_start(wv_sb[:], WprojT_r[:, :, 2 * C + hq * 256:2 * C + (hq + 1) * 256])
                        v_sb = batt.tile([P, NMC, 4, 72], BF16, name="v_sb")
                        nc.vector.memset(v_sb[:, :, :, 64:65], 1.0)
                        for mtk in range(NMC):
                            ps_v = ps_mm.tile([P, 512], F32, name="ps_v", tag="ps_mm")
                            for ch in range(CH):
                                nc.tensor.matmul(ps_v[:, 0:256], h_sb[:, ch, mtk * P:(mtk + 1) * P],
                                                 wv_sb[:, ch, :], start=(ch == 0), stop=(ch == CH - 1))
                            for hh in range(4):
                                nc.vector.tensor_copy(v_sb[:, mtk, hh, 0:64],
                                                      ps_v[:, hh * 64:(hh + 1) * 64])

                        for hp in ((2 * hq,) if debug in ("ka", "qa", "p") else (2 * hq, 2 * hq + 1)):
                            wqk_sb = bw.tile([P, CH, 256], F32R, name="wqk_sb")
                            nc.sync.dma_start(wqk_sb[:, :, 0:128], WprojT_r[:, :, hp * P:(hp + 1) * P])
                            nc.sync.dma_start(wqk_sb[:, :, 128:256],
                                              WprojT_r[:, :, C + hp * P:C + (hp + 1) * P])

                            # ---- k projection + stats (2 heads on partitions) ----
                            ka0 = batt.tile([65, T], F32R, name="ka0")
                            ka1 = batt.tile([65, T], F32R, name="ka1")
                            sk8_d = dram.tile([2, T], F32, name="sk8_d")
                            ex2k_d = dram.tile([2, T], F32, name="ex2k_d")
                            for mt in range(NMT):
                                msl = slice(mt * 512, (mt + 1) * 512)
                                ps_k = ps_mm.tile([P, 512], F32, name="ps_k", tag="ps_mm")
                                for ch in range(CH):
                                    nc.tensor.matmul(ps_k[:], wqk_sb[:, ch, 128:256],
                                                     h_sb[:, ch, msl],
                                                     start=(ch == 0), stop=(ch == CH - 1))
                                k2sb = bsm.tile([P, 512], F32R, name="k2sb")
                                nc.vector.tensor_copy(k2sb[:], ps_k[:])
                                nc.vector.tensor_copy(ka0[0:64, msl], ps_k[0:64, :])
                                nc.vector.tensor_copy(ka1[0:64, msl], ps_k[64:128, :])
                                ksq = bsm.tile([P, 512], F32R, name="ksq")
                                nc.scalar.square(ksq[:], ps_k[:])
                                ps_kr = ps_row.tile([65, 512], F32, name="ps_kr", tag="ps_row")
                                nc.tensor.matmul(ps_kr[:], bo8[:], k2sb[:], start=True, stop=True)
                                ps_kr2 = ps_row.tile([65, 512], F32, name="ps_kr2", tag="ps_row")
                                nc.tensor.matmul(ps_kr2[:], bo64[:], ksq[:], start=True, stop=True)
                                # k_aug row 64 = 8*mu_k
                                nc.vector.tensor_copy(ka0[64:65, msl], ps_kr[0:1, :])
                                nc.vector.tensor_copy(ka1[64:65, msl], ps_kr[64:65, :])
                                skr = bsm.tile([65, 512], F32, name="skr")
                                nc.vector.tensor_copy(skr[0:1, :], ps_kr[0:1, :])
                                nc.vector.tensor_copy(skr[64:65, :], ps_kr[64:65, :])
                                nc.sync.dma_start(sk8_d[0:1, msl], skr[0:1, :])
                                nc.sync.dma_start(sk8_d[1:2, msl], skr[64:65, :])
                                exr = bsm.tile([65, 512], F32, name="exr")
                                nc.vector.tensor_copy(exr[0:1, :], ps_kr2[0:1, :])
                                nc.vector.tensor_copy(exr[64:65, :], ps_kr2[64:65, :])
                                nc.sync.dma_start(ex2k_d[0:1, msl], exr[0:1, :])
                                nc.sync.dma_start(ex2k_d[1:2, msl], exr[64:65, :])

                            # rk/8 in column form [P, NMC, 2] via DRAM gather
                            sk8T = bsm.tile([P, NMC, 2], F32, name="sk8T")
                            for h_ in range(2):
                                nc.sync.dma_start(sk8T[:, :, h_],
                                                  sk8_d[h_].rearrange("(mc p) -> p mc", p=P))
                            rk8 = bsm.tile([P, NMC, 2], F32, name="rk8")
                            nc.vector.tensor_scalar_mul(rk8[:], sk8T[:], 0.125)   # mu_k
                            nc.vector.tensor_tensor(rk8[:], rk8[:], rk8[:], MUL)  # mu_k^2
                            ex2kT = bsm.tile([P, NMC, 2], F32, name="ex2kT")
                            for h_ in range(2):
                                nc.sync.dma_start(ex2kT[:, :, h_],
                                                  ex2k_d[h_].rearrange("(mc p) -> p mc", p=P))
                            nc.vector.tensor_tensor(rk8[:], ex2kT[:], rk8[:], SUB)
                            nc.scalar.activation(rk8[:], rk8[:], SQRT, bias=eps_t[:], scale=1.0)
                            nc.vector.reciprocal(rk8[:], rk8[:])
                            nc.vector.tensor_scalar_mul(rk8[:], rk8[:], 0.125)    # rk/8

                            if debug == "ka":
                                nc.gpsimd.dma_start(out.rearrange("(a b) t -> a (b t)", a=P)[0:65, 0:T],
                                                    ka0[:])
                                nc.gpsimd.dma_start(out.rearrange("(a b) t -> a (b t)", a=P)[0:P, T:T + NMC * 2],
                                                    rk8[:].rearrange("p a b -> p (a b)"))
                            if not do_q:
                                continue
                            # ---- q projection + stats ----
                            nsq8 = bsm.tile([65, TQ], F32, name="nsq8")
                            ex2q = bsm.tile([65, TQ], F32, name="ex2q")
                            q2a = []
                            for nt in range(NNT):
                                nsl = slice(nt * 512, (nt + 1) * 512)
                                ps_q = ps_mm.tile([P, 512], F32, name="ps_q", tag="ps_mm")
                                for ch in range(CH):
                                    nc.tensor.matmul(ps_q[:], wqk_sb[:, ch, 0:128],
                                                     h_sb[:, ch, nsl],
                                                     start=(ch == 0), stop=(ch == CH - 1))
                                q2t = bq.tile([P, 512], F32R, name="q2t", tag="q2t")
                                nc.vector.tensor_copy(q2t[:], ps_q[:])
                                q2a.append(q2t)
                                qsq = bsm.tile([P, 512], F32R, name="qsq")
                                nc.scalar.square(qsq[:], ps_q[:])
                                ps_qr = ps_row.tile([65, 512], F32, name="ps_qr", tag="ps_row")
                                nc.tensor.matmul(ps_qr[:], bon8[:], q2t[:], start=True, stop=True)
                                ps_qr2 = ps_row.tile([65, 512], F32, name="ps_qr2", tag="ps_row")
                                nc.tensor.matmul(ps_qr2[:], bo64[:], qsq[:], start=True, stop=True)
                                nc.vector.tensor_copy(nsq8[:, nsl], ps_qr[:])
                                nc.vector.tensor_copy(ex2q[:, nsl], ps_qr2[:])

                            rq = bsm.tile([65, TQ], F32, name="rq")
                            nc.vector.tensor_tensor(rq[:], nsq8[:], nsq8[:], MUL)
                            nc.vector.tensor_scalar_mul(rq[:], rq[:], 1.0 / 64)   # mu_q^2
                            nc.vector.tensor_tensor(rq[:], ex2q[:], rq[:], SUB)
                            nc.scalar.activation(rq[:], rq[:], SQRT, bias=eps_t[0:65], scale=1.0)
                            nc.vector.reciprocal(rq[:], rq[:])
                            rq_d = dram.tile([2, TQ], F32, name="rq_d")
                            nc.sync.dma_start(rq_d[0:1, :], rq[0:1, :])
                            nc.sync.dma_start(rq_d[1:2, :], rq[64:65, :])
                            rq_bc = bsm.tile([P, TQ], F32, name="rq_bc")
                            nc.sync.dma_start(rq_bc[0:64, :],
                                              rq_d[0:1, :].to_broadcast((64, TQ)))
                            nc.sync.dma_start(rq_bc[64:128, :],
                                              rq_d[1:2, :].to_broadcast((64, TQ)))

                            qa0 = batt.tile([65, TQ], F32R, name="qa0")
                            qa1 = batt.tile([65, TQ], F32R, name="qa1")
                            for nt in range(NNT):
                                nsl = slice(nt * 512, (nt + 1) * 512)
                                nc.vector.tensor_tensor(qa0[0:64, nsl], q2a[nt][0:64, :],
                                                        rq_bc[0:64, nsl], MUL)
                                nc.vector.tensor_tensor(qa1[0:64, nsl], q2a[nt][64:128, :],
                                                        rq_bc[64:128, nsl], MUL)
                            nc.vector.tensor_tensor(qa0[64:65, :], nsq8[0:1, :], rq[0:1, :], MUL)
                            nc.vector.tensor_tensor(qa1[64:65, :], nsq8[64:65, :], rq[64:65, :], MUL)

                            if debug == "qa":
                                ofl = out.rearrange("(a b) t -> a (b t)", a=P)
                                nc.gpsimd.dma_start(ofl[0:65, 0:TQ], qa0[:])
                                nc.gpsimd.dma_start(ofl[0:P, 2048:2560], q2a[0][:])
                                nc.gpsimd.dma_start(ofl[0:65, 4096:4096 + TQ], rq[:])
                                nc.gpsimd.dma_start(ofl[0:P, 6144:6144 + TQ], rq_bc[:])
                            if not do_sc:
                                continue
                            # ---- scores + exp + o per head ----
                            for hh, (ka, qa) in enumerate(((ka0, qa0), (ka1, qa1))):
                                head = 2 * hp + hh
                                vidx = (hp % 2) * 2 + hh
                                for nt in range(NNT):
                                    nsl = slice(nt * 512, (nt + 1) * 512)
                                    if debug == "p" and not (head == 0 and nt == 0):
                                        continue
                                    ps_o = None
                                    if debug != "p":
                                        ps_o = ps_ot.tile([65, 512], F32, name="ps_o", tag="ps_ot")
                                    for mh in range(2):
                                        p_sb = bp.tile([P, NMC // 2, 512], BF16, name="p_sb")
                                        for mi in range(NMC // 2):
                                            mc = mh * (NMC // 2) + mi
                                            ps_sT = ps_mm.tile([P, 512], F32, name="ps_sT", tag="ps_mm")
                                            nc.tensor.matmul(ps_sT[:], ka[:, mc * P:(mc + 1) * P],
                                                             qa[:, nsl], start=True, stop=True)
                                            nc.scalar.activation(p_sb[:, mi, :], ps_sT[:], EXP,
                                                                 bias=0.0, scale=rk8[:, mc, hh:hh + 1])
                                        if debug == "p":
                                            nc.gpsimd.dma_start(
                                                out.rearrange("(a b) t -> a (b t)", a=P)[:, mh * 4096:(mh + 1) * 4096],
                                                p_sb[:, :, :])
                                            continue
                                        for mi in range(NMC // 2):
                                            mc = mh * (NMC // 2) + mi
                                            nc.tensor.matmul(ps_o[:], v_sb[:, mc, vidx, 0:65],
                                                             p_sb[:, mi, :],
                                                             start=(mc == 0), stop=(mc == NMC - 1))
                                    if debug == "p":
                                        continue
                                    rec = bsm.tile([1, 512], F32, name="rec")
                                    nc.vector.reciprocal(rec[:], ps_o[64:65, :])
                                    rec_d = dram.tile([1, 512], F32, name="rec_d")
                                    nc.sync.dma_start(rec_d[:], rec[:])
                                    rec_bc = bsm.tile([64, 512], F32, name="rec_bc")
                                    nc.sync.dma_start(rec_bc[:], rec_d[:].to_broadcast((64, 512)))
                                    nc.vector.tensor_tensor(
                                        o_fm[(head % 2) * 64:(head % 2) * 64 + 64, head // 2, nsl],
                                        ps_o[0:64, :], rec_bc[:], MUL)

                if debug == "b1":
                    nc.gpsimd.dma_start(out.rearrange("(ch p) t -> p ch t", p=P),
                                        o_fm[:, :, :])
                # ============ Phase C: out = o_fm.T @ (I + W_out).T ============
                if debug is None:
                  with tc.tile_pool(name="cw", bufs=2) as cw:
                    for jt in range(C // 512):
                        wres_sb = cw.tile([P, CH, 512], F32R, name="wres_sb")
                        nc.sync.dma_start(wres_sb[:], WresT_r[:, :, jt * 512:(jt + 1) * 512])
                        for ns in range(TQ // P):
                            ps_f = ps_mm.tile([P, 512], F32, name="ps_f", tag="ps_mm")
                            for cg in range(CH):
                                nc.tensor.matmul(ps_f[:], o_fm[:, cg, ns * P:(ns + 1) * P],
                                                 wres_sb[:, cg, :],
                                                 start=(cg == 0), stop=(cg == CH - 1))
                            f_sb = cw.tile([P, 512], F32, name="f_sb")
                            nc.vector.tensor_copy(f_sb[:], ps_f[:])
                            nc.sync.dma_start(out[ns * P:(ns + 1) * P, jt * 512:(jt + 1) * 512],
                                              f_sb[:])

    nc.finalize()
    return nc


def _prep_host(x, emb, W_emb, b_emb, W_proj, W_out):
    W_embT = np.ascontiguousarray(W_emb.T.astype(np.float32))
    W_projT = np.ascontiguousarray(W_proj.T.astype(np.float32))
    W_resT = np.ascontiguousarray((np.eye(C, dtype=np.float32) + W_out).T.astype(np.float32))
    bemb2 = b_emb.astype(np.float32).copy()
    bemb2[:C] += 1.0                       # fold the FiLM "+1" into the bias
    bemb_col = np.ascontiguousarray(bemb2.reshape(O2 // P, P).T)

    in_maps = []
    for c in range(NCORES):
        b, j = c // 2, c % 2
        perm = np.concatenate([np.arange(j * TQ, (j + 1) * TQ),
                               np.arange((1 - j) * TQ, (2 - j) * TQ)])
        in_maps.append({
            "xT": np.ascontiguousarray(x[b][perm].T.astype(np.float32)),
            "embT": np.ascontiguousarray(emb[b][perm].T.astype(np.float32)),
            "WembT": W_embT, "bemb": bemb_col,
            "WprojT": W_projT, "WresT": W_resT,
        })
    return in_maps


def kernel(x, emb, W_emb, b_emb, W_proj, W_out, _trace=False, _tmpdir=None):
    x = np.asarray(x); emb = np.asarray(emb)
    W_emb = np.asarray(W_emb); b_emb = np.asarray(b_emb)
    W_proj = np.asarray(W_proj); W_out = np.asarray(W_out)

    if "nc" not in _cached:
        _cached["nc"] = build_kernel()
    nc = _cached["nc"]

    in_maps = _prep_host(x, emb, W_emb, b_emb, W_proj, W_out)
    res = run_bass_kernel_spmd(nc, in_maps, core_ids=list(range(NCORES)), trace=_trace,
                               tmpdir=_tmpdir)
    _cached["last_result"] = res

    outp = np.empty((B, N, C), dtype=np.float32)
    for c in range(NCORES):
        b, j = c // 2, c % 2
        outp[b, j * TQ:(j + 1) * TQ, :] = res.results[c]["out"]
    return outp

